# revision 46
# baseline (speedup 1.0000x reference)
"""Trainium2 Bass kernel for BaseGCN graph Laplacian (B=4, N=4096, C=3, k=20).

Math: reference computes L = I - D^{-1/2} A D^{-1/2} with A the one-hot
scatter of the k=20 nearest neighbours (euclidean, self included) per row.
top_k always returns exactly k distinct indices, so deg == k for every row
and L = I - A/k exactly: 0.95 on the diagonal, -0.05 at the 19 non-self
neighbour columns, 0 elsewhere. The diagonal is data-independent (self is
always nearest), so the host writes the exact f32 value during unshard and
the device only produces the off-diagonal -1/k pattern (plus an ignored
-1/k at the diagonal).

Sharding: 8 cores; core = 2*b + half owns rows [half*2048, half*2048+2048)
of batch b and emits a (2048, 4096) fp16 output slice; the host upcasts.

Device algorithm per 128-row chunk:
  s[i,j] = -||x_i - x_j||^2 = 2<x_i,x_j> - sq_i - sq_j via a K=24 bf16
  matmul into PSUM (three bf16 limbs per fp32 operand; error ~2^-26 x^2).
  ScalarE copies PSUM->SBUF narrowing to fp16 (11-bit mantissa: ulp near
  the threshold ~3e-5 vs a rank-20/21 gap ~4e-3 - measured 231 wrong
  entries total, rel ~6e-3; bf16's 8-bit mantissa would be ~1.7e-2).
  VectorE: per-row top-20 threshold via segmented max8 (8 segments of 512)
  -> 64 candidates -> 3 max8 rounds (the top-8 removal between rounds runs
  on GpSimd as mask+add so the DVE never stalls on its own result
  latency) -> the 20th largest value T, then the compare
  out = (s >= T) * VNEIGH as fp16->fp16 tensor_scalar (4x DVE perf mode;
  mixed-dtype variants measured 12x slower, GpSimd ~19 cyc/elem).
  DMA stores the fp16 chunk (1 MB) - half the HBM write traffic of f32.

Chunks software-pipeline: chunk c's threshold rounds and compare emit
interleaved with chunk c+1's seg scans. The DVE is the bottleneck engine
(~106us busy: 76us scan - its floor is 8.4M elems at 1/cycle/lane - plus
17us compare and ~10us rounds); ScalarE copies ~63us, GpSimd ~45us,
Tensor ~77us, DMA ~32% occupancy. Measured ~127-131us end-to-end vs the
155us baseline.
"""

import numpy as np

B, N, C = 4, 4096, 3
K = 20
P = 128                     # partition rows per chunk
ROWS = N // 2               # rows per core
NCHUNK = ROWS // P          # 16
HALF = N // 2
SEGW = 512                  # max8 segment width (aligned to PSUM banks)
NSEG = N // SEGW            # 8
NEG = -30000.0              # removal marker; must stay fp16-representable
# Match the reference's fl(dinv*dinv) rounding; the fp16 output write
# rounds it to fp16(-0.05) = -0.04998779 (1.2e-5 off).
_DINV = np.float32(1.0) / np.sqrt(np.float32(K))
VNEIGH = -float(np.float32(_DINV * _DINV))
DIAGV = float(np.float32(1.0) - np.float32(_DINV * _DINV))

_NC_CACHE = []


KMM = 24  # bf16-limb contraction depth


def _build_bass():
    import concourse.mybir as mybir
    import concourse.tile as tile
    from concourse import bacc

    f32 = mybir.dt.float32
    bf16 = mybir.dt.bfloat16
    f16 = mybir.dt.float16
    nc = bacc.Bacc("TRN2", debug=False, num_devices=8)
    rh = nc.dram_tensor("rh", (KMM, N), bf16, kind="ExternalInput").ap()
    lh = nc.dram_tensor("lh", (KMM, ROWS), bf16, kind="ExternalInput").ap()
    outp = nc.dram_tensor("outp", (ROWS, N), f16, kind="ExternalOutput").ap()

    with tile.TileContext(nc) as tc:
        with (
            tc.tile_pool(name="const", bufs=1) as const_pool,
            tc.tile_pool(name="psum", bufs=2, space="PSUM") as psum_pool,
            tc.tile_pool(name="sbig", bufs=3) as s_pool,
            tc.tile_pool(name="small", bufs=3) as small_pool,
            tc.tile_pool(name="outt", bufs=3) as out_pool,
        ):
            # Stage the input DMAs so chunk 0's first matmul (which reads
            # lh[:, :128] and rh[:, :512]) can start as soon as those small
            # pieces land, ahead of the bulk (Tile tracks sub-tile ranges).
            rh_sb = const_pool.tile([KMM, N], bf16)
            lh_sb = const_pool.tile([KMM, ROWS], bf16)
            warm = const_pool.tile([P, 8], f32)
            # Warm the Act table set (LoadActFuncSet ~1.3us) off the
            # critical path, before the first real copy needs it.
            nc.vector.memset(warm[:], 0.0)
            nc.scalar.activation(warm[:], warm[:], mybir.ActivationFunctionType.Copy)
            nc.sync.dma_start(rh_sb[:, 0:512], rh[:, 0:512])
            nc.scalar.dma_start(lh_sb[:, 0:P], lh[:, 0:P])
            nc.sync.dma_start(rh_sb[:, 512:N], rh[:, 512:N])
            nc.scalar.dma_start(lh_sb[:, P:ROWS], lh[:, P:ROWS])

            # Software pipeline: chunk c's dependent tail (3x max8 + 2x
            # match_replace rounds, then compare + DMA) is emitted
            # interleaved with chunk c+1's independent seg-max8 scans, so
            # the ~0.5us result-visibility stalls between dependent DVE ops
            # overlap useful scan work instead of idling the DVE.
            prev = None  # (s, cand, m, t1, chunk_idx, next_tail_step)
            helds = []  # chunk 13/14 tails held at step 7 for the drain

            def emit_tail_step(st):
                # Rank-20-of-64 extraction. The top-8 removal between max8
                # rounds runs on GpSimd (mask + add on 64-wide tiles, where
                # the Q7 is fine) so the DVE never sits in the ~0.5us
                # result-visibility stall between its own dependent ops;
                # the cross-engine latency pipelines across chunks. (An
                # all-DVE variant of these rounds measured ~7us slower.)
                s0, cand0, m0, t1, c0, step = st
                if step == 0:
                    nc.vector.max(m0[:, 0:8], cand0[:])
                elif step == 1:
                    # t1 = NEG where cand is in the top-8 (values are
                    # distinct: s is a continuous function of random input)
                    nc.gpsimd.tensor_scalar(
                        t1[:], cand0[:], m0[:, 7:8], NEG,
                        op0=mybir.AluOpType.is_ge, op1=mybir.AluOpType.mult,
                    )
                elif step == 2:
                    nc.gpsimd.tensor_add(cand0[:], cand0[:], t1[:])
                elif step == 3:
                    nc.vector.max(m0[:, 8:16], cand0[:])
                elif step == 4:
                    nc.gpsimd.tensor_scalar(
                        t1[:], cand0[:], m0[:, 15:16], NEG,
                        op0=mybir.AluOpType.is_ge, op1=mybir.AluOpType.mult,
                    )
                elif step == 5:
                    nc.gpsimd.tensor_add(cand0[:], cand0[:], t1[:])
                elif step == 6:
                    nc.vector.max(m0[:, 16:24], cand0[:])
                    # 20th largest value = index 19 of the sorted 24
                elif step == 7:
                    ot = out_pool.tile([P, N], f16, tag="ot")
                    nc.vector.tensor_scalar(
                        ot[:],
                        s0[:],
                        m0[:, 19:20],
                        VNEIGH,
                        op0=mybir.AluOpType.is_ge,
                        op1=mybir.AluOpType.mult,
                    )
                    nc.sync.dma_start(outp[c0 * P:(c0 + 1) * P, :], ot[:])
                    return None
                return (s0, cand0, m0, t1, c0, step + 1)

            for c in range(NCHUNK):
                s = s_pool.tile([P, N], f16, tag="s")
                cand = small_pool.tile([P, NSEG * 8], f16, tag="cand")
                # f32: tensor_scalar is_ge requires a float32 scalar operand
                m = small_pool.tile([P, 24], f32, tag="m")
                t1 = small_pool.tile([P, NSEG * 8], f16, tag="t1")
                for h in range(2):
                    ps = psum_pool.tile([P, HALF], f32, tag="ps")
                    for t in range(4):
                        col = h * HALF + t * 512
                        nc.tensor.matmul(
                            ps[:, t * 512:(t + 1) * 512],
                            lh_sb[:, c * P:(c + 1) * P],
                            rh_sb[:, col:col + 512],
                            start=True,
                            stop=True,
                        )
                        if c < 2:
                            # Head: bank-sized copy right behind each matmul
                            # so the first seg-max8s start ~3us earlier and
                            # the scan pipeline ramps without gaps.
                            g = h * 4 + t
                            nc.scalar.activation(
                                s[:, g * SEGW:(g + 1) * SEGW],
                                ps[:, t * 512:(t + 1) * 512],
                                mybir.ActivationFunctionType.Copy,
                            )
                            nc.vector.max(
                                cand[:, g * 8:(g + 1) * 8],
                                s[:, g * SEGW:(g + 1) * SEGW],
                            )
                            if c == 1 and prev is not None:
                                prev = emit_tail_step(prev)
                    if c >= 2:
                        nc.scalar.activation(
                            s[:, h * HALF:(h + 1) * HALF],
                            ps[:],
                            mybir.ActivationFunctionType.Copy,
                        )
                        # this half's 4 seg scans, with the previous chunk's
                        # tail steps woven between them
                        for g in range(h * 4, h * 4 + 4):
                            nc.vector.max(
                                cand[:, g * 8:(g + 1) * 8],
                                s[:, g * SEGW:(g + 1) * SEGW],
                            )
                            if prev is not None and not (
                                c >= NCHUNK - 2 and prev[5] >= 7
                            ):
                                # during the last two chunks, hold the
                                # previous chunk's compare (step 7) for the
                                # drain, where its halves cover the final
                                # rounds' dependency stalls
                                prev = emit_tail_step(prev)
                if c < NCHUNK - 2:
                    assert prev is None, "tail of chunk c-1 not fully drained"
                else:
                    helds.append(prev)
                prev = (s, cand, m, t1, c, 0)

            # Drain. The held chunk-13/14 compares are split in halves
            # and woven between chunk-15's dependent rounds ops: each half
            # (~0.6us of DVE work) covers a ~0.5us result-visibility or
            # cross-engine gap that would otherwise stall the DVE on the
            # kernel's critical tail.
            s13, _, m13, _, c13, _ = helds[0]
            s14, _, m14, _, c14, _ = helds[1]
            s0, cand0, m0, t10, c0, _ = prev

            ot13 = out_pool.tile([P, N], f16, tag="ot")
            ot14 = out_pool.tile([P, N], f16, tag="ot")

            def held_cmp(oth, sh, mh, ch, lo, hi, eng):
                nc.vector.tensor_scalar(
                    oth[:, lo:hi], sh[:, lo:hi], mh[:, 19:20], VNEIGH,
                    op0=mybir.AluOpType.is_ge, op1=mybir.AluOpType.mult,
                )
                eng.dma_start(outp[ch * P:(ch + 1) * P, lo:hi], oth[:, lo:hi])

            nc.vector.max(m0[:, 0:8], cand0[:])
            held_cmp(ot13, s13, m13, c13, 0, HALF, nc.sync)
            nc.gpsimd.tensor_scalar(
                t10[:], cand0[:], m0[:, 7:8], NEG,
                op0=mybir.AluOpType.is_ge, op1=mybir.AluOpType.mult,
            )
            held_cmp(ot13, s13, m13, c13, HALF, N, nc.scalar)
            nc.gpsimd.tensor_add(cand0[:], cand0[:], t10[:])
            nc.vector.max(m0[:, 8:16], cand0[:])
            held_cmp(ot14, s14, m14, c14, 0, HALF, nc.sync)
            nc.gpsimd.tensor_scalar(
                t10[:], cand0[:], m0[:, 15:16], NEG,
                op0=mybir.AluOpType.is_ge, op1=mybir.AluOpType.mult,
            )
            held_cmp(ot14, s14, m14, c14, HALF, N, nc.scalar)
            nc.gpsimd.tensor_add(cand0[:], cand0[:], t10[:])
            nc.vector.max(m0[:, 16:24], cand0[:])
            ot = out_pool.tile([P, N], f16, tag="ot")
            dma_engs = [nc.sync, nc.scalar, nc.sync, nc.scalar]
            for pi, (p0, pw) in enumerate(
                [(0, 1024), (1024, 1024), (2048, 1024), (3072, 1024)]
            ):
                qs = slice(p0, p0 + pw)
                nc.vector.tensor_scalar(
                    ot[:, qs],
                    s0[:, qs],
                    m0[:, 19:20],
                    VNEIGH,
                    op0=mybir.AluOpType.is_ge,
                    op1=mybir.AluOpType.mult,
                )
                dma_engs[pi].dma_start(outp[c0 * P:(c0 + 1) * P, qs], ot[:, qs])
    nc.compile()
    return nc


def _split3(v):
    """Split fp32 array into three bf16 limbs: v ~= h + m + l (24 bits)."""
    import ml_dtypes

    bf = ml_dtypes.bfloat16
    h = v.astype(bf)
    r = (v - h.astype(np.float32)).astype(np.float32)
    m = r.astype(bf)
    l = (r - m.astype(np.float32)).astype(bf)
    return h, m, l


def _make_in_maps(x):
    import ml_dtypes

    bf = ml_dtypes.bfloat16
    in_maps = []
    for core in range(8):
        b, half = divmod(core, 2)
        xb = x[b]                                            # (N, C)
        sq = (xb * xb).sum(axis=1, dtype=np.float32)
        rows = slice(half * ROWS, (half + 1) * ROWS)
        rh = np.empty((KMM, N), bf)
        lhs = np.empty((KMM, ROWS), bf)
        for c in range(3):
            h, m, l = _split3(xb[:, c])
            h2 = (2.0 * h.astype(np.float32)).astype(bf)
            m2 = (2.0 * m.astype(np.float32)).astype(bf)
            l2 = (2.0 * l.astype(np.float32)).astype(bf)
            # product pairs (lhs, rhs): (2h,h) (2h,m) (2m,h) (2m,m) (2h,l) (2l,h)
            rh[6 * c + 0] = h
            rh[6 * c + 1] = m
            rh[6 * c + 2] = h
            rh[6 * c + 3] = m
            rh[6 * c + 4] = l
            rh[6 * c + 5] = h
            lhs[6 * c + 0] = h2[rows]
            lhs[6 * c + 1] = h2[rows]
            lhs[6 * c + 2] = m2[rows]
            lhs[6 * c + 3] = m2[rows]
            lhs[6 * c + 4] = h2[rows]
            lhs[6 * c + 5] = l2[rows]
        sh, sm, sl = _split3(sq)
        # -sq_j rows: lhs = -1, rhs = sq limbs
        rh[18], rh[19], rh[20] = sh, sm, sl
        lhs[18] = lhs[19] = lhs[20] = np.array(-1.0, bf)
        # -sq_i rows: lhs = -sq limbs, rhs = 1
        rh[21] = rh[22] = rh[23] = np.array(1.0, bf)
        lhs[21] = (-sh.astype(np.float32)).astype(bf)[rows]
        lhs[22] = (-sm.astype(np.float32)).astype(bf)[rows]
        lhs[23] = (-sl.astype(np.float32)).astype(bf)[rows]
        in_maps.append({"rh": rh, "lh": lhs})
    return in_maps


def _ensure_trace_safe():
    """run_bass_kernel_spmd(trace=True) (e.g. env BASS_TRACE=1) needs
    antenv.axon_hooks, which some images lack, and an artifact upload that
    needs bucket access. Stub both so a traced run degrades instead of
    crashing; with tracing off these are unused."""
    import sys
    import types

    try:
        import antenv.axon_hooks  # noqa: F401
    except Exception:
        m = types.ModuleType("antenv.axon_hooks")
        m._H = None
        m.set_axon_ntff_profile_hook = lambda h: setattr(m, "_H", h)
        m.get_axon_ntff_profile_hook = lambda: m._H
        sys.modules["antenv.axon_hooks"] = m
        try:
            import antenv

            antenv.axon_hooks = m
        except Exception:
            pass


def kernel(x, k):
    x = np.ascontiguousarray(np.asarray(x), dtype=np.float32)
    k = int(np.asarray(k))
    assert x.shape == (B, N, C), f"unexpected x shape {x.shape}"
    assert k == K, f"kernel compiled for k={K}, got {k}"

    _ensure_trace_safe()
    from concourse.bass_utils import run_bass_kernel_spmd

    if not _NC_CACHE:
        _NC_CACHE.append(_build_bass())
    nc = _NC_CACHE[0]
    res = run_bass_kernel_spmd(nc, _make_in_maps(x), core_ids=list(range(8)))
    kernel.last_results = res
    out = np.empty((B, N, N), np.float32)
    for core in range(8):
        b, half = divmod(core, 2)
        out[b, half * ROWS:(half + 1) * ROWS] = res.results[core]["outp"].astype(
            np.float32
        )
    # Diagonal of L is data-independent: self is always its own nearest
    # neighbour, so L_ii = 1 - 1/k exactly; write the exact f32 value.
    idx = np.arange(N)
    out[:, idx, idx] = np.float32(DIAGV)
    return out


# revision 48
# speedup vs baseline: 1.1813x; 1.1813x over previous
"""Trainium2 Bass kernel for BaseGCN graph Laplacian (B=4, N=4096, C=3, k=20).

Math: reference computes L = I - D^{-1/2} A D^{-1/2} with A the one-hot
scatter of the k=20 nearest neighbours (euclidean, self included) per row.
top_k always returns exactly k distinct indices, so deg == k for every row
and L = I - A/k exactly: 0.95 on the diagonal, -0.05 at the 19 non-self
neighbour columns, 0 elsewhere. The diagonal is data-independent (self is
always nearest), so the host writes the exact f32 value during unshard and
the device only produces the off-diagonal -1/k pattern (plus an ignored
-1/k at the diagonal).

Sharding: 8 cores; core = 2*b + half owns rows [half*2048, half*2048+2048)
of batch b and emits a (2048, 4096) fp16 output slice; the host upcasts.

Device algorithm per 128-row chunk:
  s[i,j] = -||x_i - x_j||^2 = 2<x_i,x_j> - sq_i - sq_j via a K=24 bf16
  matmul into PSUM (three bf16 limbs per fp32 operand; error ~2^-26 x^2).
  ScalarE copies PSUM->SBUF narrowing to fp16 (11-bit mantissa: ulp near
  the threshold ~3e-5 vs a rank-20/21 gap ~4e-3 - measured 231 wrong
  entries total, rel ~6e-3; bf16's 8-bit mantissa would be ~1.7e-2).
  VectorE: per-row top-20 threshold via segmented max8 (8 segments of 512)
  -> 64 candidates -> 3 max8 rounds (the top-8 removal between rounds runs
  on GpSimd as mask+add so the DVE never stalls on its own result
  latency) -> the 20th largest value T, then the compare
  out = (s >= T) * VNEIGH as fp16->fp16 tensor_scalar (4x DVE perf mode;
  mixed-dtype variants measured 12x slower, GpSimd ~19 cyc/elem).
  DMA stores the fp16 chunk (1 MB) - half the HBM write traffic of f32.

Chunks software-pipeline: chunk c's threshold rounds and compare emit
interleaved with chunk c+1's seg scans; on the kernel tail, chunk 14's
compare is held for the drain and its halves are woven between chunk 15's
dependent rounds ops so they cover the ~0.5us result-visibility gaps
(ABBA-verified ~0.8us faster than a plain drain). The DVE is the
bottleneck engine (~106us busy: 76us scan - its floor is 8.4M elems at
1/cycle/lane since max8 has only a 1x uop - plus 17us compare and ~10us
rounds); ScalarE copies ~63us, GpSimd ~45us, Tensor ~77us, DMA ~32%
occupancy. Measured 126.9-127.7us end-to-end vs the 155us baseline.
"""

import numpy as np

B, N, C = 4, 4096, 3
K = 20
P = 128                     # partition rows per chunk
ROWS = N // 2               # rows per core
NCHUNK = ROWS // P          # 16
HALF = N // 2
SEGW = 512                  # max8 segment width (aligned to PSUM banks)
NSEG = N // SEGW            # 8
NEG = -30000.0              # removal marker; must stay fp16-representable
# Match the reference's fl(dinv*dinv) rounding; the fp16 output write
# rounds it to fp16(-0.05) = -0.04998779 (1.2e-5 off).
_DINV = np.float32(1.0) / np.sqrt(np.float32(K))
VNEIGH = -float(np.float32(_DINV * _DINV))
DIAGV = float(np.float32(1.0) - np.float32(_DINV * _DINV))

_NC_CACHE = []


KMM = 24  # bf16-limb contraction depth


def _build_bass():
    import concourse.mybir as mybir
    import concourse.tile as tile
    from concourse import bacc

    f32 = mybir.dt.float32
    bf16 = mybir.dt.bfloat16
    f16 = mybir.dt.float16
    nc = bacc.Bacc("TRN2", debug=False, num_devices=8)
    rh = nc.dram_tensor("rh", (KMM, N), bf16, kind="ExternalInput").ap()
    lh = nc.dram_tensor("lh", (KMM, ROWS), bf16, kind="ExternalInput").ap()
    outp = nc.dram_tensor("outp", (ROWS, N), f16, kind="ExternalOutput").ap()

    with tile.TileContext(nc) as tc:
        with (
            tc.tile_pool(name="const", bufs=1) as const_pool,
            tc.tile_pool(name="psum", bufs=2, space="PSUM") as psum_pool,
            tc.tile_pool(name="sbig", bufs=3) as s_pool,
            tc.tile_pool(name="small", bufs=3) as small_pool,
            tc.tile_pool(name="outt", bufs=3) as out_pool,
        ):
            # Stage the input DMAs so chunk 0's first matmul (which reads
            # lh[:, :128] and rh[:, :512]) can start as soon as those small
            # pieces land, ahead of the bulk (Tile tracks sub-tile ranges).
            rh_sb = const_pool.tile([KMM, N], bf16)
            lh_sb = const_pool.tile([KMM, ROWS], bf16)
            warm = const_pool.tile([P, 8], f32)
            # Warm the Act table set (LoadActFuncSet ~1.3us) off the
            # critical path, before the first real copy needs it.
            nc.vector.memset(warm[:], 0.0)
            nc.scalar.activation(warm[:], warm[:], mybir.ActivationFunctionType.Copy)
            nc.sync.dma_start(rh_sb[:, 0:512], rh[:, 0:512])
            nc.scalar.dma_start(lh_sb[:, 0:P], lh[:, 0:P])
            nc.sync.dma_start(rh_sb[:, 512:N], rh[:, 512:N])
            nc.scalar.dma_start(lh_sb[:, P:ROWS], lh[:, P:ROWS])

            # Software pipeline: chunk c's dependent tail (3x max8 + 2x
            # match_replace rounds, then compare + DMA) is emitted
            # interleaved with chunk c+1's independent seg-max8 scans, so
            # the ~0.5us result-visibility stalls between dependent DVE ops
            # overlap useful scan work instead of idling the DVE.
            prev = None  # (s, cand, m, t1, chunk_idx, next_tail_step)

            def emit_tail_step(st):
                # Rank-20-of-64 extraction. The top-8 removal between max8
                # rounds runs on GpSimd (mask + add on 64-wide tiles, where
                # the Q7 is fine) so the DVE never sits in the ~0.5us
                # result-visibility stall between its own dependent ops;
                # the cross-engine latency pipelines across chunks. (An
                # all-DVE variant of these rounds measured ~7us slower.)
                s0, cand0, m0, t1, c0, step = st
                if step == 0:
                    nc.vector.max(m0[:, 0:8], cand0[:])
                elif step == 1:
                    # t1 = NEG where cand is in the top-8 (values are
                    # distinct: s is a continuous function of random input)
                    nc.gpsimd.tensor_scalar(
                        t1[:], cand0[:], m0[:, 7:8], NEG,
                        op0=mybir.AluOpType.is_ge, op1=mybir.AluOpType.mult,
                    )
                elif step == 2:
                    nc.gpsimd.tensor_add(cand0[:], cand0[:], t1[:])
                elif step == 3:
                    nc.vector.max(m0[:, 8:16], cand0[:])
                elif step == 4:
                    nc.gpsimd.tensor_scalar(
                        t1[:], cand0[:], m0[:, 15:16], NEG,
                        op0=mybir.AluOpType.is_ge, op1=mybir.AluOpType.mult,
                    )
                elif step == 5:
                    nc.gpsimd.tensor_add(cand0[:], cand0[:], t1[:])
                elif step == 6:
                    nc.vector.max(m0[:, 16:24], cand0[:])
                    # 20th largest value = index 19 of the sorted 24
                elif step == 7:
                    ot = out_pool.tile([P, N], f16, tag="ot")
                    nc.vector.tensor_scalar(
                        ot[:],
                        s0[:],
                        m0[:, 19:20],
                        VNEIGH,
                        op0=mybir.AluOpType.is_ge,
                        op1=mybir.AluOpType.mult,
                    )
                    nc.sync.dma_start(outp[c0 * P:(c0 + 1) * P, :], ot[:])
                    return None
                return (s0, cand0, m0, t1, c0, step + 1)

            for c in range(NCHUNK):
                s = s_pool.tile([P, N], f16, tag="s")
                cand = small_pool.tile([P, NSEG * 8], f16, tag="cand")
                # f32: tensor_scalar is_ge requires a float32 scalar operand
                m = small_pool.tile([P, 24], f32, tag="m")
                t1 = small_pool.tile([P, NSEG * 8], f16, tag="t1")
                for h in range(2):
                    ps = psum_pool.tile([P, HALF], f32, tag="ps")
                    for t in range(4):
                        col = h * HALF + t * 512
                        nc.tensor.matmul(
                            ps[:, t * 512:(t + 1) * 512],
                            lh_sb[:, c * P:(c + 1) * P],
                            rh_sb[:, col:col + 512],
                            start=True,
                            stop=True,
                        )
                        if c < 2:
                            # Head: bank-sized copy right behind each matmul
                            # so the first seg-max8s start ~3us earlier and
                            # the scan pipeline ramps without gaps.
                            g = h * 4 + t
                            nc.scalar.activation(
                                s[:, g * SEGW:(g + 1) * SEGW],
                                ps[:, t * 512:(t + 1) * 512],
                                mybir.ActivationFunctionType.Copy,
                            )
                            nc.vector.max(
                                cand[:, g * 8:(g + 1) * 8],
                                s[:, g * SEGW:(g + 1) * SEGW],
                            )
                            if c == 1 and prev is not None:
                                prev = emit_tail_step(prev)
                    if c >= 2:
                        nc.scalar.activation(
                            s[:, h * HALF:(h + 1) * HALF],
                            ps[:],
                            mybir.ActivationFunctionType.Copy,
                        )
                        # this half's 4 seg scans, with the previous chunk's
                        # tail steps woven between them
                        for g in range(h * 4, h * 4 + 4):
                            nc.vector.max(
                                cand[:, g * 8:(g + 1) * 8],
                                s[:, g * SEGW:(g + 1) * SEGW],
                            )
                            if prev is not None and not (
                                c == NCHUNK - 1 and prev[5] >= 7
                            ):
                                # during the last chunk, hold the previous
                                # chunk's compare (step 7) for the drain,
                                # where it covers the final rounds' stalls
                                prev = emit_tail_step(prev)
                if c < NCHUNK - 1:
                    assert prev is None, "tail of chunk c-1 not fully drained"
                else:
                    held = prev
                prev = (s, cand, m, t1, c, 0)

            # Drain. The held chunk-14 compare is split in halves and woven
            # between chunk-15's dependent rounds ops: each half (~0.6us of
            # DVE work) covers a ~0.5us result-visibility/cross-engine gap
            # that would otherwise stall the DVE on the critical tail.
            s14, _, m14, _, c14, _ = held
            s0, cand0, m0, t10, c0, _ = prev
            ot14 = out_pool.tile([P, N], f16, tag="ot")
            nc.vector.max(m0[:, 0:8], cand0[:])
            nc.vector.tensor_scalar(
                ot14[:, 0:HALF], s14[:, 0:HALF], m14[:, 19:20], VNEIGH,
                op0=mybir.AluOpType.is_ge, op1=mybir.AluOpType.mult,
            )
            nc.sync.dma_start(outp[c14 * P:(c14 + 1) * P, 0:HALF], ot14[:, 0:HALF])
            nc.gpsimd.tensor_scalar(
                t10[:], cand0[:], m0[:, 7:8], NEG,
                op0=mybir.AluOpType.is_ge, op1=mybir.AluOpType.mult,
            )
            nc.vector.tensor_scalar(
                ot14[:, HALF:N], s14[:, HALF:N], m14[:, 19:20], VNEIGH,
                op0=mybir.AluOpType.is_ge, op1=mybir.AluOpType.mult,
            )
            nc.scalar.dma_start(outp[c14 * P:(c14 + 1) * P, HALF:N], ot14[:, HALF:N])
            nc.gpsimd.tensor_add(cand0[:], cand0[:], t10[:])
            nc.vector.max(m0[:, 8:16], cand0[:])
            nc.vector.tensor_scalar(
                t10[:], cand0[:], m0[:, 15:16], NEG,
                op0=mybir.AluOpType.is_ge, op1=mybir.AluOpType.mult,
            )
            nc.vector.tensor_tensor(
                cand0[:], cand0[:], t10[:], op=mybir.AluOpType.add
            )
            nc.vector.max(m0[:, 16:24], cand0[:])
            ot = out_pool.tile([P, N], f16, tag="ot")
            dma_engs = [nc.sync, nc.scalar, nc.sync, nc.scalar]
            for pi, (p0, pw) in enumerate(
                [(0, 1024), (1024, 1024), (2048, 1024), (3072, 1024)]
            ):
                qs = slice(p0, p0 + pw)
                nc.vector.tensor_scalar(
                    ot[:, qs],
                    s0[:, qs],
                    m0[:, 19:20],
                    VNEIGH,
                    op0=mybir.AluOpType.is_ge,
                    op1=mybir.AluOpType.mult,
                )
                dma_engs[pi].dma_start(outp[c0 * P:(c0 + 1) * P, qs], ot[:, qs])
    nc.compile()
    return nc


def _split3(v):
    """Split fp32 array into three bf16 limbs: v ~= h + m + l (24 bits)."""
    import ml_dtypes

    bf = ml_dtypes.bfloat16
    h = v.astype(bf)
    r = (v - h.astype(np.float32)).astype(np.float32)
    m = r.astype(bf)
    l = (r - m.astype(np.float32)).astype(bf)
    return h, m, l


def _make_in_maps(x):
    import ml_dtypes

    bf = ml_dtypes.bfloat16
    in_maps = []
    for core in range(8):
        b, half = divmod(core, 2)
        xb = x[b]                                            # (N, C)
        sq = (xb * xb).sum(axis=1, dtype=np.float32)
        rows = slice(half * ROWS, (half + 1) * ROWS)
        rh = np.empty((KMM, N), bf)
        lhs = np.empty((KMM, ROWS), bf)
        for c in range(3):
            h, m, l = _split3(xb[:, c])
            h2 = (2.0 * h.astype(np.float32)).astype(bf)
            m2 = (2.0 * m.astype(np.float32)).astype(bf)
            l2 = (2.0 * l.astype(np.float32)).astype(bf)
            # product pairs (lhs, rhs): (2h,h) (2h,m) (2m,h) (2m,m) (2h,l) (2l,h)
            rh[6 * c + 0] = h
            rh[6 * c + 1] = m
            rh[6 * c + 2] = h
            rh[6 * c + 3] = m
            rh[6 * c + 4] = l
            rh[6 * c + 5] = h
            lhs[6 * c + 0] = h2[rows]
            lhs[6 * c + 1] = h2[rows]
            lhs[6 * c + 2] = m2[rows]
            lhs[6 * c + 3] = m2[rows]
            lhs[6 * c + 4] = h2[rows]
            lhs[6 * c + 5] = l2[rows]
        sh, sm, sl = _split3(sq)
        # -sq_j rows: lhs = -1, rhs = sq limbs
        rh[18], rh[19], rh[20] = sh, sm, sl
        lhs[18] = lhs[19] = lhs[20] = np.array(-1.0, bf)
        # -sq_i rows: lhs = -sq limbs, rhs = 1
        rh[21] = rh[22] = rh[23] = np.array(1.0, bf)
        lhs[21] = (-sh.astype(np.float32)).astype(bf)[rows]
        lhs[22] = (-sm.astype(np.float32)).astype(bf)[rows]
        lhs[23] = (-sl.astype(np.float32)).astype(bf)[rows]
        in_maps.append({"rh": rh, "lh": lhs})
    return in_maps


def _ensure_trace_safe():
    """run_bass_kernel_spmd(trace=True) (e.g. env BASS_TRACE=1) needs
    antenv.axon_hooks, which some images lack, and an artifact upload that
    needs bucket access. Stub both so a traced run degrades instead of
    crashing; with tracing off these are unused."""
    import sys
    import types

    try:
        import antenv.axon_hooks  # noqa: F401
    except Exception:
        m = types.ModuleType("antenv.axon_hooks")
        m._H = None
        m.set_axon_ntff_profile_hook = lambda h: setattr(m, "_H", h)
        m.get_axon_ntff_profile_hook = lambda: m._H
        sys.modules["antenv.axon_hooks"] = m
        try:
            import antenv

            antenv.axon_hooks = m
        except Exception:
            pass


def kernel(x, k):
    x = np.ascontiguousarray(np.asarray(x), dtype=np.float32)
    k = int(np.asarray(k))
    assert x.shape == (B, N, C), f"unexpected x shape {x.shape}"
    assert k == K, f"kernel compiled for k={K}, got {k}"

    _ensure_trace_safe()
    from concourse.bass_utils import run_bass_kernel_spmd

    if not _NC_CACHE:
        _NC_CACHE.append(_build_bass())
    nc = _NC_CACHE[0]
    res = run_bass_kernel_spmd(nc, _make_in_maps(x), core_ids=list(range(8)))
    kernel.last_results = res
    out = np.empty((B, N, N), np.float32)
    for core in range(8):
        b, half = divmod(core, 2)
        out[b, half * ROWS:(half + 1) * ROWS] = res.results[core]["outp"].astype(
            np.float32
        )
    # Diagonal of L is data-independent: self is always its own nearest
    # neighbour, so L_ii = 1 - 1/k exactly; write the exact f32 value.
    idx = np.arange(N)
    out[:, idx, idx] = np.float32(DIAGV)
    return out
